# revision 76
# baseline (speedup 1.0000x reference)
"""FCOS head (nn_FCOS_73787538145418) Trainium2 Bass kernel.

Sharding: data-parallel, one image per NeuronCore (B=8 across 8 cores),
weights replicated. Each core runs the identical SPMD NEFF over its image.

Stem convs (two 4-layer 3x3 conv 256->256 + ReLU stems per level) run as
1D Winograd F(2,3) along x: per output-column pair, 4 Winograd points
m0..m3 accumulate in separate PSUM banks over (2 ci chunks x 3 ky taps)
K=128 matmuls on bf16 transformed inputs/weights (1.5x fewer PE rows than
direct conv), then y0=relu(m0+m1+m2+b), y1=relu(m1-m2-m3+b) recombine on
the vector/gpsimd/scalar engines while the PE streams the next band.
Weight transform happens on the host; p3 features ship as bf16 to halve
the startup-critical DMA. Prediction convs (cls 20ch; box+ctr 5ch) stay
direct fp32r (18 PSUM-accumulated matmuls) — a Winograd pred costs the
same input-transform engine work for a 4x smaller matmul window and
starves the PE. Output is [25, 5376] channel-major per core; the host
transposes and stacks to (8, 5376, 25).

Hard-won scheduling facts: tensor_tensor may read at most ONE PSUM
operand; gpsimd cannot touch PSUM at all and runs TT at ~half DVE rate;
DMA writes to one tile are WAW-ordered across queues (ship them on one
queue in need-order); weight pool tiles must be DMA'd on the sync queue
(scalar-queue DMAs into pool tiles corrupt); per-matmul issue overhead
is ~26ns (bf16 near zero), LDWEIGHTS hides under M>=256 streams; fp32r
matmuls drop to 1/4 rate below M=256 and 1/2 rate out of top p-state,
so keeping the PE gapless compounds.
"""
import sys

if '/opt/trn_rl_repo' not in sys.path:
    sys.path.insert(0, '/opt/trn_rl_repo')

import numpy as np
import ml_dtypes

import concourse.mybir as mybir
from concourse import bacc
import concourse.tile as tile
from concourse.bass_utils import run_bass_kernel_spmd

P = 128
NCH = 2                 # 256 channels = 2 chunks of 128
C = 256
NL = 4                  # stem depth
NPIX_TOTAL = 5376
F32R = mybir.dt.float32r
F32 = mybir.dt.float32
BF16 = mybir.dt.bfloat16
ADD = mybir.AluOpType.add
SUB = mybir.AluOpType.subtract
RELU = mybir.ActivationFunctionType.Relu

_cached = {}
_run_opts = {}   # extra kwargs for run_bass_kernel_spmd (test harness: trace)
_last = {}       # last BassKernelResults (test harness reads exec_time_ns)


def _pad_view(flat_tile, off, H, W):
    n = NCH * (H + 2) * (W + 2)
    return flat_tile[:, off:off + n].rearrange(
        "p (c h w) -> p c h w", c=NCH, h=H + 2, w=W + 2)


def _pair_view(flat_tile, off, H, W):
    # stays f32r-typed: relu writes must be f32r-rounded for the fp32r
    # prediction-conv matmuls that consume the towers
    n = NCH * (H + 2) * (W + 2)
    return flat_tile[:, off:off + n].rearrange(
        "p (c h x two) -> p c h x two",
        c=NCH, h=H + 2, x=(W + 2) // 2, two=2)


def _zero_ring(nc, v, H, W):
    f = v.bitcast(F32)
    for c in range(NCH):
        nc.vector.memset(f[:, c, 0, :], 0.0)
        nc.vector.memset(f[:, c, H + 1, :], 0.0)
        nc.vector.memset(f[:, c, 1:H + 1, 0], 0.0)
        nc.vector.memset(f[:, c, 1:H + 1, W + 1], 0.0)


class _WL:
    """One Winograd stem conv layer (3x3 same, 256->256, + bias + ReLU)."""

    def __init__(self, nc, wpool, upool, spool, psum, vw_d, sbias,
                 s, l, src_pv, dst_pv, H, W, tag, fine_tf=False):
        self.nc = nc
        self.wpool, self.upool, self.spool, self.psum = \
            wpool, upool, spool, psum
        self.vw_d, self.sbias = vw_d, sbias
        self.s, self.l = s, l
        self.src, self.dst = src_pv, dst_pv
        self.H, self.W, self.tag = H, W, tag
        self.fine_tf = fine_tf
        self.RB = 16 if H == 64 else H      # band rows
        self.NB = H // self.RB              # bands
        self.TX = W // 2                    # tiles per row
        self.RR = self.RB + 2               # U rows per band
        self.wts = {}
        self.us = {}
        self.ms = {}

    def weights_pt(self, pt, eng=None):
        if pt in self.wts:
            return
        nc = self.nc
        eng = eng or nc.sync
        wt = self.wpool.tile([P, NCH, NCH, 3, P], BF16, tag="ww",
                             name=f"ww_{self.tag}_{pt}")
        eng.dma_start(wt[:], self.vw_d[self.s, self.l, pt])
        self.wts[pt] = wt

    def weights(self, eng=None):
        for pt in range(4):
            self.weights_pt(pt, eng)

    def tf(self, q):
        """Input transform for band q: U[c, r, pt, tx] (bf16).

        One op per Winograd point covering both ci chunks; pt0 on the
        vector engine, pt1-3 on gpsimd (vector carries the output
        transform, gpsimd is otherwise idle)."""
        if q in self.us:
            return
        nc = self.nc
        TX = self.TX
        u = self.upool.tile([P, NCH, self.RR, 4, TX], BF16, tag="u",
                            name=f"u_{self.tag}_{q}")
        r0 = self.RB * q
        rows = slice(r0, r0 + self.RR)
        if self.fine_tf:
            # startup-critical: smaller ops, spread over both engines
            for c in range(NCH):
                e0 = self.src[:, c, rows, 0:TX, 0]
                e1 = self.src[:, c, rows, 1:TX + 1, 0]
                o0 = self.src[:, c, rows, 0:TX, 1]
                o1 = self.src[:, c, rows, 1:TX + 1, 1]
                nc.vector.tensor_tensor(u[:, c, :, 0], e0, e1, SUB)
                nc.vector.tensor_tensor(u[:, c, :, 1], o0, e1, ADD)
                nc.gpsimd.tensor_tensor(u[:, c, :, 2], e1, o0, SUB)
                nc.gpsimd.tensor_tensor(u[:, c, :, 3], o0, o1, SUB)
        else:
            e0 = self.src[:, :, rows, 0:TX, 0]
            e1 = self.src[:, :, rows, 1:TX + 1, 0]
            o0 = self.src[:, :, rows, 0:TX, 1]
            o1 = self.src[:, :, rows, 1:TX + 1, 1]
            nc.vector.tensor_tensor(u[:, :, :, 0], e0, e1, SUB)
            nc.gpsimd.tensor_tensor(u[:, :, :, 1], o0, e1, ADD)
            nc.gpsimd.tensor_tensor(u[:, :, :, 2], e1, o0, SUB)
            nc.gpsimd.tensor_tensor(u[:, :, :, 3], o0, o1, SUB)
        self.us[q] = u

    def mm(self, b):
        nc = self.nc
        u = self.us[b]
        self.ms[b] = {}
        for o in range(NCH):
            ms = [self.psum.tile([P, self.RB, self.TX], F32, tag="ps",
                                 name=f"m_{self.tag}_{b}_{o}_{pt}")
                  for pt in range(4)]
            for pt in range(4):
                wt = self.wts[pt]
                k = 0
                for c in range(NCH):
                    for ky in range(3):
                        nc.tensor.matmul(ms[pt][:], wt[:, c, o, ky],
                                         u[:, c, ky:ky + self.RB, pt],
                                         start=(k == 0), stop=(k == 5))
                        k += 1
            self.ms[b][o] = ms

    def outtf(self, b):
        """y0 = relu(m0+m1+m2+b) -> odd cols; y1 = relu(m1-m2-m3+b) -> even."""
        nc = self.nc
        RB, TX = self.RB, self.TX
        rows = slice(1 + RB * b, 1 + RB * (b + 1))
        for o in range(NCH):
            m0, m1, m2, m3 = self.ms[b][o]
            t = f"{self.tag}_{b}_{o}"
            c2 = self.spool.tile([P, RB, TX], F32, tag="sc", name=f"c2_{t}")
            tP = self.spool.tile([P, RB, TX], F32, tag="sc", name=f"tp_{t}")
            tM = self.spool.tile([P, RB, TX], F32, tag="sc", name=f"tm_{t}")
            r0 = self.spool.tile([P, RB, TX], F32, tag="sc", name=f"r0_{t}")
            r1 = self.spool.tile([P, RB, TX], F32, tag="sc", name=f"r1_{t}")
            bias = self.sbias[:, self.s, self.l, o]
            nc.scalar.copy(c2[:], m2[:])
            nc.vector.tensor_tensor(tP[:], m1[:], c2[:], ADD)
            nc.vector.tensor_tensor(tM[:], m1[:], c2[:], SUB)
            nc.vector.tensor_tensor(r0[:], m0[:], tP[:], ADD)
            nc.vector.tensor_tensor(r1[:], m3[:], tM[:], SUB)   # m3 - tM
            nc.scalar.activation(self.dst[:, o, rows, 0:TX, 1], r0[:],
                                 RELU, bias=bias)
            nc.scalar.activation(self.dst[:, o, rows, 1:TX + 1, 0], r1[:],
                                 RELU, bias=bias, scale=-1.0)
        del self.ms[b]


def _emit_chain(layers, post_hooks=None):
    """Emit a list of _WL layers sequentially with next-layer tf hoisting."""
    post_hooks = post_hooks or {}
    n = len(layers)
    for i, L in enumerate(layers):
        nxt = layers[i + 1] if i + 1 < n else None
        L.weights()
        L.tf(0)
        if L.NB > 1:
            L.tf(1)
        for b in range(L.NB):
            L.mm(b)
            if b + 2 < L.NB:
                L.tf(b + 2)
            if b == L.NB - 1 and nxt is not None:
                nxt.weights()
                nxt.tf(0)
            L.outtf(b)
            if b == L.NB - 1 and nxt is not None and nxt.NB > 1:
                nxt.tf(1)
        if i in post_hooks:
            post_hooks[i]()


def _preds_cls(nc, psum_pool, stage_pool, pwc, pbc, tower, out_d,
               H, W, R, pix_base, tag):
    n_tiles = H // R
    for it in range(n_tiles):
        rr = it * R
        ps1 = psum_pool.tile([P, R, W], F32, tag="ps", name=f"pc_{tag}_{it}")
        k = 0
        for c in range(NCH):
            for t in range(9):
                ky, kx = t // 3, t % 3
                rc = tower[:, c, rr + ky:rr + ky + R, kx:kx + W]
                nc.tensor.matmul(ps1[0:20], pwc[:, c, t], rc,
                                 start=(k == 0), stop=(k == 17))
                k += 1
        st = stage_pool.tile([32, R * W], F32, tag="st", name=f"st_{tag}_{it}")
        nc.vector.tensor_tensor(
            st[0:20], ps1[0:20].rearrange("p r w -> p (r w)"),
            pbc[:20].to_broadcast([20, R * W]), ADD)
        c0 = pix_base + rr * W
        nc.sync.dma_start(out_d[0:20, c0:c0 + R * W], st[0:20])


def _preds_box(nc, psum_pool, stage_pool, pwb, pbb, tower, out_d,
               H, W, R, pix_base, tag):
    n_tiles = H // R
    for it in range(n_tiles):
        rr = it * R
        ps2 = psum_pool.tile([P, R, W], F32, tag="ps", name=f"pb_{tag}_{it}")
        k = 0
        for c in range(NCH):
            for t in range(9):
                ky, kx = t // 3, t % 3
                rb = tower[:, c, rr + ky:rr + ky + R, kx:kx + W]
                nc.tensor.matmul(ps2[0:5], pwb[:, c, t], rb,
                                 start=(k == 0), stop=(k == 17))
                k += 1
        st = stage_pool.tile([32, R * W], F32, tag="st", name=f"s2_{tag}_{it}")
        nc.vector.tensor_tensor(
            st[0:5], ps2[0:5].rearrange("p r w -> p (r w)"),
            pbb[:5].to_broadcast([5, R * W]), ADD)
        c0 = pix_base + rr * W
        nc.sync.dma_start(out_d[20:25, c0:c0 + R * W], st[0:5])


class _PW:
    """(unused) Winograd prediction conv: the tower input transform costs
    the same engine work as a stem layer but the matmul window is 4x
    shorter (20/5 output channels), so gpsimd can't keep up and the PE
    starves. Kept for reference; direct f32r preds win.

    wv: sbuf tile [P, 4, NCH, 3, n_out] bf16; bias_ap: [n_out, 1];
    writes out_d[ch0:ch0+n_out, pix_base + ...]."""

    def __init__(self, nc, psum, upool, spool, stage_pool, wv, bias_ap,
                 n_out, ch0, tower_pv, out_d, H, W, pix_base, tag):
        self.nc, self.psum, self.upool, self.spool, self.stage = \
            nc, psum, upool, spool, stage_pool
        self.wv, self.bias_ap, self.n_out, self.ch0 = wv, bias_ap, n_out, ch0
        self.src, self.out_d, self.pix = tower_pv, out_d, pix_base
        self.H, self.W, self.tag = H, W, tag
        self.RB = 16 if H == 64 else H
        self.NB = H // self.RB
        self.TX = W // 2
        self.RR = self.RB + 2
        self.us = {}
        self.ms = {}

    def tf(self, q):
        if q in self.us:
            return
        nc = self.nc
        TX = self.TX
        u = self.upool.tile([P, NCH, self.RR, 4, TX], BF16, tag="u",
                            name=f"up_{self.tag}_{q}")
        rows = slice(self.RB * q, self.RB * q + self.RR)
        e0 = self.src[:, :, rows, 0:TX, 0]
        e1 = self.src[:, :, rows, 1:TX + 1, 0]
        o0 = self.src[:, :, rows, 0:TX, 1]
        o1 = self.src[:, :, rows, 1:TX + 1, 1]
        nc.vector.tensor_tensor(u[:, :, :, 0], e0, e1, SUB)
        nc.gpsimd.tensor_tensor(u[:, :, :, 1], o0, e1, ADD)
        nc.gpsimd.tensor_tensor(u[:, :, :, 2], e1, o0, SUB)
        nc.gpsimd.tensor_tensor(u[:, :, :, 3], o0, o1, SUB)
        self.us[q] = u

    def mm(self, b):
        nc = self.nc
        u = self.us[b]
        ms = [self.psum.tile([P, self.RB, self.TX], F32, tag="ps",
                             name=f"mp_{self.tag}_{b}_{pt}")
              for pt in range(4)]
        for pt in range(4):
            k = 0
            for c in range(NCH):
                for ky in range(3):
                    nc.tensor.matmul(ms[pt][0:self.n_out],
                                     self.wv[:, pt, c, ky],
                                     u[:, c, ky:ky + self.RB, pt],
                                     start=(k == 0), stop=(k == 5))
                    k += 1
        self.ms[b] = ms

    def outtf(self, b):
        nc = self.nc
        RB, TX, n = self.RB, self.TX, self.n_out
        m0, m1, m2, m3 = [m[0:n] for m in self.ms[b]]
        t = f"{self.tag}_{b}"
        c2 = self.spool.tile([P, RB, TX], F32, tag="sc", name=f"pc2_{t}")
        tP = self.spool.tile([P, RB, TX], F32, tag="sc", name=f"ptp_{t}")
        tM = self.spool.tile([P, RB, TX], F32, tag="sc", name=f"ptm_{t}")
        st = self.stage.tile([32, RB, TX, 2], F32, tag="st", name=f"st_{t}")
        nc.scalar.copy(c2[0:n], m2)
        nc.vector.tensor_tensor(tP[0:n], m1, c2[0:n], ADD)
        # tM = m1 + bias - m2
        nc.vector.scalar_tensor_tensor(tM[0:n], m1, self.bias_ap,
                                       c2[0:n], ADD, SUB)
        # y0 = m0 + bias + (m1+m2)
        nc.vector.scalar_tensor_tensor(st[0:n, :, :, 0], m0, self.bias_ap,
                                       tP[0:n], ADD, ADD)
        # y1 = (m3 * -1) + tM = m1 - m2 - m3 + bias
        nc.vector.scalar_tensor_tensor(st[0:n, :, :, 1], m3, -1.0,
                                       tM[0:n], mybir.AluOpType.mult, ADD)
        c0 = self.pix + self.RB * b * self.W
        nc.sync.dma_start(
            self.out_d[self.ch0:self.ch0 + n, c0:c0 + RB * self.W],
            st[0:n].rearrange("p r x t -> p (r x t)"))
        del self.ms[b]

    def emit(self):
        self.tf(0)
        if self.NB > 1:
            self.tf(1)
        for b in range(self.NB):
            self.mm(b)
            if b + 2 < self.NB:
                self.tf(b + 2)
            self.outtf(b)


def _build():
    nc = bacc.Bacc("TRN2", target_bir_lowering=False, debug=False,
                   num_devices=8)

    # p3 feat ships bf16 (it only feeds winograd transforms) — halves the
    # startup-critical DMA; p4/p5 feats stay f32r in pad0
    x_d = [nc.dram_tensor("x0", (P, NCH, 66, 66), BF16,
                          kind="ExternalInput"),
           nc.dram_tensor("x1", (P, NCH, 34, 34), F32R,
                          kind="ExternalInput"),
           nc.dram_tensor("x2", (P, NCH, 18, 18), F32R,
                          kind="ExternalInput")]
    vw_d = nc.dram_tensor("vw", (2, NL, 4, P, NCH, NCH, 3, P), BF16,
                          kind="ExternalInput")
    sb_d = nc.dram_tensor("sb", (2, NL, NCH, P, 1), F32, kind="ExternalInput")
    pwc_d = nc.dram_tensor("pwc", (P, NCH, 9, 20), F32R, kind="ExternalInput")
    pwb_d = nc.dram_tensor("pwb", (P, NCH, 9, 5), F32R, kind="ExternalInput")
    pbc_d = nc.dram_tensor("pbc", (20, 1), F32, kind="ExternalInput")
    pbb_d = nc.dram_tensor("pbb", (5, 1), F32, kind="ExternalInput")
    out_d = nc.dram_tensor("out", (25, NPIX_TOTAL), F32, kind="ExternalOutput")

    N3 = NCH * 66 * 66            # 8712: p3 padded elems/partition
    N4 = NCH * 34 * 34            # 2312
    N5 = NCH * 18 * 18            # 648
    PAD0 = N3 + N4 + N5           # pad0 also hosts the p4/p5 feat regions

    with tile.TileContext(nc) as tc:
        with (
            tc.tile_pool(name="resident", bufs=1) as res_pool,
            tc.tile_pool(name="wwts", bufs=8) as wwts_pool,
            tc.tile_pool(name="upool", bufs=3) as upool,
            tc.tile_pool(name="scratch", bufs=8) as spool,
            tc.tile_pool(name="psum", bufs=8, space="PSUM") as psum_pool,
            tc.tile_pool(name="stage", bufs=4) as stage_pool,
        ):
            pad0 = res_pool.tile([P, PAD0], F32R, name="pad0")
            pad1 = res_pool.tile([P, N3], F32R, name="pad1")
            pad2 = res_pool.tile([P, N3], F32R, name="pad2")
            feat3 = res_pool.tile([P, NCH, 66, 33, 2], BF16, name="feat3")

            sbias = res_pool.tile([P, 2, NL, NCH, 1], F32, name="sbias")
            pwc = res_pool.tile([P, NCH, 9, 20], F32R, name="pwc")
            pwb = res_pool.tile([P, NCH, 9, 5], F32R, name="pwb")
            pbc = res_pool.tile([32, 1], F32, name="pbc")
            pbb = res_pool.tile([32, 1], F32, name="pbb")

            # fp32r padded views (pred conv moving operands) + fp32 pair
            # views (winograd transforms / relu writes) per rotation buffer.
            A3r, A3 = _pad_view(pad0, 0, 64, 64), _pair_view(pad0, 0, 64, 64)
            B3r, B3 = _pad_view(pad1, 0, 64, 64), _pair_view(pad1, 0, 64, 64)
            C3 = _pair_view(pad2, 0, 64, 64)
            A4r, A4 = _pad_view(pad0, N3, 32, 32), _pair_view(pad0, N3, 32, 32)
            B4r, B4 = _pad_view(pad1, 0, 32, 32), _pair_view(pad1, 0, 32, 32)
            C4r, C4 = _pad_view(pad2, 0, 32, 32), _pair_view(pad2, 0, 32, 32)
            A5r, A5 = (_pad_view(pad0, N3 + N4, 16, 16),
                       _pair_view(pad0, N3 + N4, 16, 16))
            B5r, B5 = (_pad_view(pad1, N4, 16, 16),
                       _pair_view(pad1, N4, 16, 16))
            C5r, C5 = (_pad_view(pad2, N4, 16, 16),
                       _pair_view(pad2, N4, 16, 16))

            def wl(s, l, src, dst, H, W, tag, fine_tf=False):
                return _WL(nc, wwts_pool, upool, spool, psum_pool, vw_d,
                           sbias, s, l, src, dst, H, W, tag, fine_tf)

            # p3 scratch rings (A3 never holds the DMA'd feat now, so its
            # ring needs zeroing too)
            _zero_ring(nc, _pad_view(pad0, 0, 64, 64), 64, 64)
            _zero_ring(nc, _pad_view(pad1, 0, 64, 64), 64, 64)
            _zero_ring(nc, _pad_view(pad2, 0, 64, 64), 64, 64)

            # ---- startup DMAs ----
            # consts on the scalar queue (scalar is idle early); the p3
            # back-rows follow (emitted below), then p4/p5 feats
            nc.scalar.dma_start(
                sbias[:],
                sb_d[:].rearrange("s l a p o -> p (s l a o)")
                       .rearrange("p (s l a o) -> p s l a o",
                                  s=2, l=NL, a=NCH))
            nc.scalar.dma_start(pwc[:], pwc_d[:])
            nc.scalar.dma_start(pwb[:], pwb_d[:])
            nc.scalar.dma_start(pbc[:20], pbc_d[:])
            nc.scalar.dma_start(pbb[:5], pbb_d[:])

            # p3 pass: cls l0 F->B, box l0 F->C, cls B->A->B->A (tower A),
            # box C->B->C->B (tower B)
            F3 = feat3[:]
            p3 = [wl(0, 0, F3, B3, 64, 64, "a00", fine_tf=True),
                  wl(1, 0, F3, C3, 64, 64, "a10"),
                  wl(0, 1, B3, A3, 64, 64, "a01"),
                  wl(0, 2, A3, B3, 64, 64, "a02"),
                  wl(0, 3, B3, A3, 64, 64, "a03"),
                  wl(1, 1, C3, B3, 64, 64, "a11"),
                  wl(1, 2, B3, C3, 64, 64, "a12"),
                  wl(1, 3, C3, B3, 64, 64, "a13")]

            # first wino layer weights (pt0 first) interleaved with the p3
            # feature bands so the first matmuls' deps clear early
            # coarse startup DMAs: descriptor ISSUE costs ~650ns each on the
            # queue engine, so fewer/bigger transfers win at startup
            def _feat_rows(r0, r1, eng):
                eng.dma_start(feat3[:, :, r0:r1], x_d[0][:, :, r0:r1])

            # All pad0-feeding DMAs share the sync queue in ascending-need
            # order: DMA writes to one tile are WAW-ordered across queues,
            # so spreading them over queues serializes anyway (and in
            # emission order, which can invert the need order). Weight pool
            # tiles also corrupt if DMA'd off the sync queue, so everything
            # big lives here; weights slot in after the first two row
            # chunks (which feed the first two band transforms).
            _feat_rows(0, 18, nc.sync)
            p3[0].weights_pt(0)
            p3[0].weights_pt(1)
            _feat_rows(18, 34, nc.sync)
            _feat_rows(34, 50, nc.sync)
            p3[0].weights()
            _feat_rows(50, 66, nc.sync)
            nc.sync.dma_start(A4r[:, :], x_d[1][:])
            nc.sync.dma_start(A5r[:, :], x_d[2][:])

            def clspred3():
                _preds_cls(nc, psum_pool, stage_pool, pwc, pbc, A3r, out_d,
                           64, 64, 8, 0, "a")

            _emit_chain(p3, post_hooks={5: clspred3})

            # p4/p5 pass: cls l0 A->B, box l0 A->C, then in-place
            # (towers: cls=B, box=C); p4/p5 layers interleaved for slack
            p4 = [wl(0, 0, A4, B4, 32, 32, "b00"),
                  wl(1, 0, A4, C4, 32, 32, "b10"),
                  wl(0, 1, B4, B4, 32, 32, "b01"),
                  wl(1, 1, C4, C4, 32, 32, "b11"),
                  wl(0, 2, B4, B4, 32, 32, "b02"),
                  wl(1, 2, C4, C4, 32, 32, "b12"),
                  wl(0, 3, B4, B4, 32, 32, "b03"),
                  wl(1, 3, C4, C4, 32, 32, "b13")]
            p5 = [wl(0, 0, A5, B5, 16, 16, "c00"),
                  wl(1, 0, A5, C5, 16, 16, "c10"),
                  wl(0, 1, B5, B5, 16, 16, "c01"),
                  wl(1, 1, C5, C5, 16, 16, "c11"),
                  wl(0, 2, B5, B5, 16, 16, "c02"),
                  wl(1, 2, C5, C5, 16, 16, "c12"),
                  wl(0, 3, B5, B5, 16, 16, "c03"),
                  wl(1, 3, C5, C5, 16, 16, "c13")]
            # first p4/p5 layers' weights + input transforms hide under the
            # p3 box-pred matmuls; their feats were loaded mid-pass-A
            p4[0].weights()
            p5[0].weights()
            p4[0].tf(0)
            p5[0].tf(0)
            _preds_box(nc, psum_pool, stage_pool, pwb, pbb, B3r, out_d,
                       64, 64, 8, 0, "a")

            # p4/p5 scratch rings (pad1/pad2 free after p3 preds)
            _zero_ring(nc, _pad_view(pad1, 0, 32, 32), 32, 32)
            _zero_ring(nc, _pad_view(pad2, 0, 32, 32), 32, 32)
            _zero_ring(nc, _pad_view(pad1, N4, 16, 16), 16, 16)
            _zero_ring(nc, _pad_view(pad2, N4, 16, 16), 16, 16)

            for i in range(8):
                for L in (p4[i], p5[i]):
                    L.weights()
                    L.tf(0)
                    L.mm(0)
                    L.outtf(0)

            _preds_cls(nc, psum_pool, stage_pool, pwc, pbc, B4r, out_d,
                       32, 32, 16, 4096, "b")
            _preds_cls(nc, psum_pool, stage_pool, pwc, pbc, B5r, out_d,
                       16, 16, 16, 5120, "c")
            _preds_box(nc, psum_pool, stage_pool, pwb, pbb, C4r, out_d,
                       32, 32, 16, 4096, "b")
            _preds_box(nc, psum_pool, stage_pool, pwb, pbb, C5r, out_d,
                       16, 16, 16, 5120, "c")

    nc.compile()
    return nc


def _pack_wino_w(wcls, wbox):
    # [s, l, co, ci, ky, kx] -> wino V [s, l, pt, cip, cic, coc, ky, cop]
    w = np.stack([wcls, wbox]).astype(np.float32)   # [2, NL, 256, 256, 3, 3]
    V = np.stack([w[..., 0],
                  (w[..., 0] + w[..., 1] + w[..., 2]) * 0.5,
                  (w[..., 0] - w[..., 1] + w[..., 2]) * 0.5,
                  w[..., 2]], axis=-1)              # [2, NL, co, ci, ky, pt]
    V = V.reshape(2, NL, NCH, P, NCH, P, 3, 4)      # [s,l,coc,cop,cic,cip,ky,pt]
    V = V.transpose(0, 1, 7, 5, 4, 2, 6, 3)         # [s,l,pt,cip,cic,coc,ky,cop]
    return np.ascontiguousarray(V).astype(ml_dtypes.bfloat16)


def _pack_pred_w(w):
    # [co, ci, ky, kx] -> [cip, cic, tap, co]
    n = w.shape[0]
    w = np.asarray(w, np.float32).reshape(n, NCH, P, 3, 3)
    w = w.transpose(2, 1, 3, 4, 0)
    return np.ascontiguousarray(w.reshape(P, NCH, 9, n), dtype=np.float32)


def kernel(p3, p4, p5, stem_cls_w, stem_cls_b, stem_box_w, stem_box_b,
           pred_cls_w, pred_cls_b, pred_box_w, pred_box_b,
           pred_ctr_w, pred_ctr_b):
    if 'nc' not in _cached:
        _cached['nc'] = _build()
    nc = _cached['nc']

    B = p3.shape[0]
    vw = _pack_wino_w(np.asarray(stem_cls_w), np.asarray(stem_box_w))
    sb = np.ascontiguousarray(
        np.stack([stem_cls_b, stem_box_b]).reshape(2, NL, NCH, P, 1),
        dtype=np.float32)
    pwc = _pack_pred_w(np.asarray(pred_cls_w))
    pwb = _pack_pred_w(np.concatenate([pred_box_w, pred_ctr_w], axis=0))
    pbc = np.asarray(pred_cls_b, np.float32).reshape(20, 1)
    pbb = np.concatenate([pred_box_b, pred_ctr_b]).astype(np.float32).reshape(5, 1)

    shared = {"vw": vw, "sb": sb, "pwc": pwc, "pwb": pwb,
              "pbc": pbc, "pbb": pbb}
    xs = [np.asarray(p3, np.float32), np.asarray(p4, np.float32),
          np.asarray(p5, np.float32)]
    in_maps = []
    for b in range(B):
        m = dict(shared)
        for i, x in enumerate(xs):
            xp = np.pad(x[b].reshape(NCH, P, x.shape[2], x.shape[3]),
                        ((0, 0), (0, 0), (1, 1), (1, 1)))
            xp = np.ascontiguousarray(xp.transpose(1, 0, 2, 3))
            m[f"x{i}"] = xp.astype(ml_dtypes.bfloat16) if i == 0 else xp
        in_maps.append(m)

    res = run_bass_kernel_spmd(nc, in_maps, core_ids=list(range(B)),
                               **_run_opts)
    _last['res'] = res
    out = np.stack([r["out"].T for r in res.results])
    return np.ascontiguousarray(out, dtype=np.float32)


# revision 79
# speedup vs baseline: 1.0058x; 1.0058x over previous
"""FCOS head (nn_FCOS_73787538145418) Trainium2 Bass kernel.

Sharding: data-parallel, one image per NeuronCore (B=8 across 8 cores),
weights replicated. Each core runs the identical SPMD NEFF over its image.

Stem convs (two 4-layer 3x3 conv 256->256 + ReLU stems per level) run as
1D Winograd F(2,3) along x: per output-column pair, 4 Winograd points
m0..m3 accumulate in separate PSUM banks over (2 ci chunks x 3 ky taps)
K=128 matmuls on bf16 transformed inputs/weights (1.5x fewer PE rows than
direct conv), then y0=relu(m0+m1+m2+b), y1=relu(m1-m2-m3+b) recombine on
the vector/gpsimd/scalar engines while the PE streams the next band.
Weight transform happens on the host; p3 features ship as bf16 to halve
the startup-critical DMA. Prediction convs (cls 20ch; box+ctr 5ch) stay
direct fp32r (18 PSUM-accumulated matmuls) — a Winograd pred costs the
same input-transform engine work for a 4x smaller matmul window and
starves the PE. Output is [25, 5376] channel-major per core; the host
transposes and stacks to (8, 5376, 25).

Hard-won scheduling facts: tensor_tensor may read at most ONE PSUM
operand; gpsimd cannot touch PSUM at all and runs TT at ~half DVE rate;
DMA writes to one tile are WAW-ordered across queues (ship them on one
queue in need-order); weight pool tiles must be DMA'd on the sync queue
(scalar-queue DMAs into pool tiles corrupt); per-matmul issue overhead
is ~26ns (bf16 near zero), LDWEIGHTS hides under M>=256 streams; fp32r
matmuls drop to 1/4 rate below M=256 and 1/2 rate out of top p-state,
so keeping the PE gapless compounds.
"""
import sys

if '/opt/trn_rl_repo' not in sys.path:
    sys.path.insert(0, '/opt/trn_rl_repo')

import numpy as np
import ml_dtypes

import concourse.mybir as mybir
from concourse import bacc
import concourse.tile as tile
from concourse.bass_utils import run_bass_kernel_spmd

P = 128
NCH = 2                 # 256 channels = 2 chunks of 128
C = 256
NL = 4                  # stem depth
NPIX_TOTAL = 5376
F32R = mybir.dt.float32r
F32 = mybir.dt.float32
BF16 = mybir.dt.bfloat16
ADD = mybir.AluOpType.add
SUB = mybir.AluOpType.subtract
RELU = mybir.ActivationFunctionType.Relu

_cached = {}
_run_opts = {}   # extra kwargs for run_bass_kernel_spmd (test harness: trace)
_last = {}       # last BassKernelResults (test harness reads exec_time_ns)


def _pad_view(flat_tile, off, H, W):
    n = NCH * (H + 2) * (W + 2)
    return flat_tile[:, off:off + n].rearrange(
        "p (c h w) -> p c h w", c=NCH, h=H + 2, w=W + 2)


def _pair_view(flat_tile, off, H, W):
    # stays f32r-typed: relu writes must be f32r-rounded for the fp32r
    # prediction-conv matmuls that consume the towers
    n = NCH * (H + 2) * (W + 2)
    return flat_tile[:, off:off + n].rearrange(
        "p (c h x two) -> p c h x two",
        c=NCH, h=H + 2, x=(W + 2) // 2, two=2)


def _zero_ring(nc, v, H, W):
    f = v.bitcast(F32)
    for c in range(NCH):
        nc.vector.memset(f[:, c, 0, :], 0.0)
        nc.vector.memset(f[:, c, H + 1, :], 0.0)
        nc.vector.memset(f[:, c, 1:H + 1, 0], 0.0)
        nc.vector.memset(f[:, c, 1:H + 1, W + 1], 0.0)


class _WL:
    """One Winograd stem conv layer (3x3 same, 256->256, + bias + ReLU)."""

    def __init__(self, nc, wpool, upool, spool, psum, vw_d, sbias,
                 s, l, src_pv, dst_pv, H, W, tag, fine_tf=False):
        self.nc = nc
        self.wpool, self.upool, self.spool, self.psum = \
            wpool, upool, spool, psum
        self.vw_d, self.sbias = vw_d, sbias
        self.s, self.l = s, l
        self.src, self.dst = src_pv, dst_pv
        self.H, self.W, self.tag = H, W, tag
        self.fine_tf = fine_tf
        self.RB = 16 if H == 64 else H      # band rows
        self.NB = H // self.RB              # bands
        self.TX = W // 2                    # tiles per row
        self.RR = self.RB + 2               # U rows per band
        self.wts = {}
        self.us = {}
        self.ms = {}

    def weights_pt(self, pt, eng=None):
        if pt in self.wts:
            return
        nc = self.nc
        eng = eng or nc.sync
        wt = self.wpool.tile([P, NCH, NCH, 3, P], BF16, tag="ww",
                             name=f"ww_{self.tag}_{pt}")
        eng.dma_start(wt[:], self.vw_d[self.s, self.l, pt])
        self.wts[pt] = wt

    def weights(self, eng=None):
        for pt in range(4):
            self.weights_pt(pt, eng)

    def tf(self, q):
        """Input transform for band q: U[c, r, pt, tx] (bf16).

        One op per Winograd point covering both ci chunks; pt0 on the
        vector engine, pt1-3 on gpsimd (vector carries the output
        transform, gpsimd is otherwise idle)."""
        if q in self.us:
            return
        nc = self.nc
        TX = self.TX
        u = self.upool.tile([P, NCH, self.RR, 4, TX], BF16, tag="u",
                            name=f"u_{self.tag}_{q}")
        r0 = self.RB * q
        rows = slice(r0, r0 + self.RR)
        if self.fine_tf:
            # startup-critical: smaller ops, spread over both engines
            for c in range(NCH):
                e0 = self.src[:, c, rows, 0:TX, 0]
                e1 = self.src[:, c, rows, 1:TX + 1, 0]
                o0 = self.src[:, c, rows, 0:TX, 1]
                o1 = self.src[:, c, rows, 1:TX + 1, 1]
                nc.vector.tensor_tensor(u[:, c, :, 0], e0, e1, SUB)
                nc.vector.tensor_tensor(u[:, c, :, 1], o0, e1, ADD)
                nc.vector.tensor_tensor(u[:, c, :, 2], e1, o0, SUB)
                nc.gpsimd.tensor_tensor(u[:, c, :, 3], o0, o1, SUB)
        else:
            e0 = self.src[:, :, rows, 0:TX, 0]
            e1 = self.src[:, :, rows, 1:TX + 1, 0]
            o0 = self.src[:, :, rows, 0:TX, 1]
            o1 = self.src[:, :, rows, 1:TX + 1, 1]
            nc.vector.tensor_tensor(u[:, :, :, 0], e0, e1, SUB)
            nc.gpsimd.tensor_tensor(u[:, :, :, 1], o0, e1, ADD)
            nc.gpsimd.tensor_tensor(u[:, :, :, 2], e1, o0, SUB)
            nc.gpsimd.tensor_tensor(u[:, :, :, 3], o0, o1, SUB)
        self.us[q] = u

    def mm(self, b):
        nc = self.nc
        u = self.us[b]
        self.ms[b] = {}
        for o in range(NCH):
            ms = [self.psum.tile([P, self.RB, self.TX], F32, tag="ps",
                                 name=f"m_{self.tag}_{b}_{o}_{pt}")
                  for pt in range(4)]
            for pt in range(4):
                wt = self.wts[pt]
                k = 0
                for c in range(NCH):
                    for ky in range(3):
                        nc.tensor.matmul(ms[pt][:], wt[:, c, o, ky],
                                         u[:, c, ky:ky + self.RB, pt],
                                         start=(k == 0), stop=(k == 5))
                        k += 1
            self.ms[b][o] = ms

    def outtf(self, b):
        """y0 = relu(m0+m1+m2+b) -> odd cols; y1 = relu(m1-m2-m3+b) -> even."""
        nc = self.nc
        RB, TX = self.RB, self.TX
        rows = slice(1 + RB * b, 1 + RB * (b + 1))
        for o in range(NCH):
            m0, m1, m2, m3 = self.ms[b][o]
            t = f"{self.tag}_{b}_{o}"
            c2 = self.spool.tile([P, RB, TX], F32, tag="sc", name=f"c2_{t}")
            tP = self.spool.tile([P, RB, TX], F32, tag="sc", name=f"tp_{t}")
            tM = self.spool.tile([P, RB, TX], F32, tag="sc", name=f"tm_{t}")
            r0 = self.spool.tile([P, RB, TX], F32, tag="sc", name=f"r0_{t}")
            r1 = self.spool.tile([P, RB, TX], F32, tag="sc", name=f"r1_{t}")
            bias = self.sbias[:, self.s, self.l, o]
            nc.scalar.copy(c2[:], m2[:])
            nc.vector.tensor_tensor(tP[:], m1[:], c2[:], ADD)
            nc.vector.tensor_tensor(tM[:], m1[:], c2[:], SUB)
            nc.vector.tensor_tensor(r0[:], m0[:], tP[:], ADD)
            nc.vector.tensor_tensor(r1[:], m3[:], tM[:], SUB)   # m3 - tM
            nc.scalar.activation(self.dst[:, o, rows, 0:TX, 1], r0[:],
                                 RELU, bias=bias)
            nc.scalar.activation(self.dst[:, o, rows, 1:TX + 1, 0], r1[:],
                                 RELU, bias=bias, scale=-1.0)
        del self.ms[b]


def _emit_chain(layers, post_hooks=None):
    """Emit a list of _WL layers sequentially with next-layer tf hoisting."""
    post_hooks = post_hooks or {}
    n = len(layers)
    for i, L in enumerate(layers):
        nxt = layers[i + 1] if i + 1 < n else None
        L.weights()
        L.tf(0)
        if L.NB > 1:
            L.tf(1)
        for b in range(L.NB):
            L.mm(b)
            if b + 2 < L.NB:
                L.tf(b + 2)
            if b == L.NB - 1 and nxt is not None:
                nxt.weights()
                nxt.tf(0)
            L.outtf(b)
            if b == L.NB - 1 and nxt is not None and nxt.NB > 1:
                nxt.tf(1)
        if i in post_hooks:
            post_hooks[i]()


def _preds_cls(nc, psum_pool, stage_pool, pwc, pbc, tower, out_d,
               H, W, R, pix_base, tag):
    n_tiles = H // R
    for it in range(n_tiles):
        rr = it * R
        ps1 = psum_pool.tile([P, R, W], F32, tag="ps", name=f"pc_{tag}_{it}")
        k = 0
        for c in range(NCH):
            for t in range(9):
                ky, kx = t // 3, t % 3
                rc = tower[:, c, rr + ky:rr + ky + R, kx:kx + W]
                nc.tensor.matmul(ps1[0:20], pwc[:, c, t], rc,
                                 start=(k == 0), stop=(k == 17))
                k += 1
        st = stage_pool.tile([32, R * W], F32, tag="st", name=f"st_{tag}_{it}")
        nc.vector.tensor_tensor(
            st[0:20], ps1[0:20].rearrange("p r w -> p (r w)"),
            pbc[:20].to_broadcast([20, R * W]), ADD)
        c0 = pix_base + rr * W
        nc.sync.dma_start(out_d[0:20, c0:c0 + R * W], st[0:20])


def _preds_box(nc, psum_pool, stage_pool, pwb, pbb, tower, out_d,
               H, W, R, pix_base, tag):
    n_tiles = H // R
    for it in range(n_tiles):
        rr = it * R
        ps2 = psum_pool.tile([P, R, W], F32, tag="ps", name=f"pb_{tag}_{it}")
        k = 0
        for c in range(NCH):
            for t in range(9):
                ky, kx = t // 3, t % 3
                rb = tower[:, c, rr + ky:rr + ky + R, kx:kx + W]
                nc.tensor.matmul(ps2[0:5], pwb[:, c, t], rb,
                                 start=(k == 0), stop=(k == 17))
                k += 1
        st = stage_pool.tile([32, R * W], F32, tag="st", name=f"s2_{tag}_{it}")
        nc.vector.tensor_tensor(
            st[0:5], ps2[0:5].rearrange("p r w -> p (r w)"),
            pbb[:5].to_broadcast([5, R * W]), ADD)
        c0 = pix_base + rr * W
        nc.sync.dma_start(out_d[20:25, c0:c0 + R * W], st[0:5])


class _PW:
    """(unused) Winograd prediction conv: the tower input transform costs
    the same engine work as a stem layer but the matmul window is 4x
    shorter (20/5 output channels), so gpsimd can't keep up and the PE
    starves. Kept for reference; direct f32r preds win.

    wv: sbuf tile [P, 4, NCH, 3, n_out] bf16; bias_ap: [n_out, 1];
    writes out_d[ch0:ch0+n_out, pix_base + ...]."""

    def __init__(self, nc, psum, upool, spool, stage_pool, wv, bias_ap,
                 n_out, ch0, tower_pv, out_d, H, W, pix_base, tag):
        self.nc, self.psum, self.upool, self.spool, self.stage = \
            nc, psum, upool, spool, stage_pool
        self.wv, self.bias_ap, self.n_out, self.ch0 = wv, bias_ap, n_out, ch0
        self.src, self.out_d, self.pix = tower_pv, out_d, pix_base
        self.H, self.W, self.tag = H, W, tag
        self.RB = 16 if H == 64 else H
        self.NB = H // self.RB
        self.TX = W // 2
        self.RR = self.RB + 2
        self.us = {}
        self.ms = {}

    def tf(self, q):
        if q in self.us:
            return
        nc = self.nc
        TX = self.TX
        u = self.upool.tile([P, NCH, self.RR, 4, TX], BF16, tag="u",
                            name=f"up_{self.tag}_{q}")
        rows = slice(self.RB * q, self.RB * q + self.RR)
        e0 = self.src[:, :, rows, 0:TX, 0]
        e1 = self.src[:, :, rows, 1:TX + 1, 0]
        o0 = self.src[:, :, rows, 0:TX, 1]
        o1 = self.src[:, :, rows, 1:TX + 1, 1]
        nc.vector.tensor_tensor(u[:, :, :, 0], e0, e1, SUB)
        nc.gpsimd.tensor_tensor(u[:, :, :, 1], o0, e1, ADD)
        nc.gpsimd.tensor_tensor(u[:, :, :, 2], e1, o0, SUB)
        nc.gpsimd.tensor_tensor(u[:, :, :, 3], o0, o1, SUB)
        self.us[q] = u

    def mm(self, b):
        nc = self.nc
        u = self.us[b]
        ms = [self.psum.tile([P, self.RB, self.TX], F32, tag="ps",
                             name=f"mp_{self.tag}_{b}_{pt}")
              for pt in range(4)]
        for pt in range(4):
            k = 0
            for c in range(NCH):
                for ky in range(3):
                    nc.tensor.matmul(ms[pt][0:self.n_out],
                                     self.wv[:, pt, c, ky],
                                     u[:, c, ky:ky + self.RB, pt],
                                     start=(k == 0), stop=(k == 5))
                    k += 1
        self.ms[b] = ms

    def outtf(self, b):
        nc = self.nc
        RB, TX, n = self.RB, self.TX, self.n_out
        m0, m1, m2, m3 = [m[0:n] for m in self.ms[b]]
        t = f"{self.tag}_{b}"
        c2 = self.spool.tile([P, RB, TX], F32, tag="sc", name=f"pc2_{t}")
        tP = self.spool.tile([P, RB, TX], F32, tag="sc", name=f"ptp_{t}")
        tM = self.spool.tile([P, RB, TX], F32, tag="sc", name=f"ptm_{t}")
        st = self.stage.tile([32, RB, TX, 2], F32, tag="st", name=f"st_{t}")
        nc.scalar.copy(c2[0:n], m2)
        nc.vector.tensor_tensor(tP[0:n], m1, c2[0:n], ADD)
        # tM = m1 + bias - m2
        nc.vector.scalar_tensor_tensor(tM[0:n], m1, self.bias_ap,
                                       c2[0:n], ADD, SUB)
        # y0 = m0 + bias + (m1+m2)
        nc.vector.scalar_tensor_tensor(st[0:n, :, :, 0], m0, self.bias_ap,
                                       tP[0:n], ADD, ADD)
        # y1 = (m3 * -1) + tM = m1 - m2 - m3 + bias
        nc.vector.scalar_tensor_tensor(st[0:n, :, :, 1], m3, -1.0,
                                       tM[0:n], mybir.AluOpType.mult, ADD)
        c0 = self.pix + self.RB * b * self.W
        nc.sync.dma_start(
            self.out_d[self.ch0:self.ch0 + n, c0:c0 + RB * self.W],
            st[0:n].rearrange("p r x t -> p (r x t)"))
        del self.ms[b]

    def emit(self):
        self.tf(0)
        if self.NB > 1:
            self.tf(1)
        for b in range(self.NB):
            self.mm(b)
            if b + 2 < self.NB:
                self.tf(b + 2)
            self.outtf(b)


def _build():
    nc = bacc.Bacc("TRN2", target_bir_lowering=False, debug=False,
                   num_devices=8)

    # p3 feat ships bf16 (it only feeds winograd transforms) — halves the
    # startup-critical DMA; p4/p5 feats stay f32r in pad0
    x_d = [nc.dram_tensor("x0", (P, NCH, 66, 66), BF16,
                          kind="ExternalInput"),
           nc.dram_tensor("x1", (P, NCH, 34, 34), F32R,
                          kind="ExternalInput"),
           nc.dram_tensor("x2", (P, NCH, 18, 18), F32R,
                          kind="ExternalInput")]
    vw_d = nc.dram_tensor("vw", (2, NL, 4, P, NCH, NCH, 3, P), BF16,
                          kind="ExternalInput")
    sb_d = nc.dram_tensor("sb", (2, NL, NCH, P, 1), F32, kind="ExternalInput")
    pwc_d = nc.dram_tensor("pwc", (P, NCH, 9, 20), F32R, kind="ExternalInput")
    pwb_d = nc.dram_tensor("pwb", (P, NCH, 9, 5), F32R, kind="ExternalInput")
    pbc_d = nc.dram_tensor("pbc", (20, 1), F32, kind="ExternalInput")
    pbb_d = nc.dram_tensor("pbb", (5, 1), F32, kind="ExternalInput")
    out_d = nc.dram_tensor("out", (25, NPIX_TOTAL), F32, kind="ExternalOutput")

    N3 = NCH * 66 * 66            # 8712: p3 padded elems/partition
    N4 = NCH * 34 * 34            # 2312
    N5 = NCH * 18 * 18            # 648
    PAD0 = N3 + N4 + N5           # pad0 also hosts the p4/p5 feat regions

    with tile.TileContext(nc) as tc:
        with (
            tc.tile_pool(name="resident", bufs=1) as res_pool,
            tc.tile_pool(name="wwts", bufs=8) as wwts_pool,
            tc.tile_pool(name="upool", bufs=3) as upool,
            tc.tile_pool(name="scratch", bufs=8) as spool,
            tc.tile_pool(name="psum", bufs=8, space="PSUM") as psum_pool,
            tc.tile_pool(name="stage", bufs=4) as stage_pool,
        ):
            pad0 = res_pool.tile([P, PAD0], F32R, name="pad0")
            pad1 = res_pool.tile([P, N3], F32R, name="pad1")
            pad2 = res_pool.tile([P, N3], F32R, name="pad2")
            feat3 = res_pool.tile([P, NCH, 66, 33, 2], BF16, name="feat3")

            sbias = res_pool.tile([P, 2, NL, NCH, 1], F32, name="sbias")
            pwc = res_pool.tile([P, NCH, 9, 20], F32R, name="pwc")
            pwb = res_pool.tile([P, NCH, 9, 5], F32R, name="pwb")
            pbc = res_pool.tile([32, 1], F32, name="pbc")
            pbb = res_pool.tile([32, 1], F32, name="pbb")

            # fp32r padded views (pred conv moving operands) + fp32 pair
            # views (winograd transforms / relu writes) per rotation buffer.
            A3r, A3 = _pad_view(pad0, 0, 64, 64), _pair_view(pad0, 0, 64, 64)
            B3r, B3 = _pad_view(pad1, 0, 64, 64), _pair_view(pad1, 0, 64, 64)
            C3 = _pair_view(pad2, 0, 64, 64)
            A4r, A4 = _pad_view(pad0, N3, 32, 32), _pair_view(pad0, N3, 32, 32)
            B4r, B4 = _pad_view(pad1, 0, 32, 32), _pair_view(pad1, 0, 32, 32)
            C4r, C4 = _pad_view(pad2, 0, 32, 32), _pair_view(pad2, 0, 32, 32)
            A5r, A5 = (_pad_view(pad0, N3 + N4, 16, 16),
                       _pair_view(pad0, N3 + N4, 16, 16))
            B5r, B5 = (_pad_view(pad1, N4, 16, 16),
                       _pair_view(pad1, N4, 16, 16))
            C5r, C5 = (_pad_view(pad2, N4, 16, 16),
                       _pair_view(pad2, N4, 16, 16))

            def wl(s, l, src, dst, H, W, tag, fine_tf=False):
                return _WL(nc, wwts_pool, upool, spool, psum_pool, vw_d,
                           sbias, s, l, src, dst, H, W, tag, fine_tf)

            # p3 scratch rings: B3/C3 up front; A3 (first read ~60us in) is
            # zeroed after layer 0 so the startup vector queue stays clear
            _zero_ring(nc, _pad_view(pad1, 0, 64, 64), 64, 64)
            _zero_ring(nc, _pad_view(pad2, 0, 64, 64), 64, 64)

            # ---- startup DMAs ----
            # consts on the scalar queue (scalar is idle early); the p3
            # back-rows follow (emitted below), then p4/p5 feats
            nc.scalar.dma_start(
                sbias[:],
                sb_d[:].rearrange("s l a p o -> p (s l a o)")
                       .rearrange("p (s l a o) -> p s l a o",
                                  s=2, l=NL, a=NCH))
            nc.scalar.dma_start(pwc[:], pwc_d[:])
            nc.scalar.dma_start(pwb[:], pwb_d[:])
            nc.scalar.dma_start(pbc[:20], pbc_d[:])
            nc.scalar.dma_start(pbb[:5], pbb_d[:])

            # p3 pass: cls l0 F->B, box l0 F->C, cls B->A->B->A (tower A),
            # box C->B->C->B (tower B)
            F3 = feat3[:]
            p3 = [wl(0, 0, F3, B3, 64, 64, "a00", fine_tf=True),
                  wl(1, 0, F3, C3, 64, 64, "a10"),
                  wl(0, 1, B3, A3, 64, 64, "a01"),
                  wl(0, 2, A3, B3, 64, 64, "a02"),
                  wl(0, 3, B3, A3, 64, 64, "a03"),
                  wl(1, 1, C3, B3, 64, 64, "a11"),
                  wl(1, 2, B3, C3, 64, 64, "a12"),
                  wl(1, 3, C3, B3, 64, 64, "a13")]

            # first wino layer weights (pt0 first) interleaved with the p3
            # feature bands so the first matmuls' deps clear early
            # coarse startup DMAs: descriptor ISSUE costs ~650ns each on the
            # queue engine, so fewer/bigger transfers win at startup
            def _feat_rows(r0, r1, eng):
                eng.dma_start(feat3[:, :, r0:r1], x_d[0][:, :, r0:r1])

            # All pad0-feeding DMAs share the sync queue in ascending-need
            # order: DMA writes to one tile are WAW-ordered across queues,
            # so spreading them over queues serializes anyway (and in
            # emission order, which can invert the need order). Weight pool
            # tiles also corrupt if DMA'd off the sync queue, so everything
            # big lives here; weights slot in after the first two row
            # chunks (which feed the first two band transforms).
            _feat_rows(0, 18, nc.sync)
            p3[0].weights_pt(0)
            p3[0].weights_pt(1)
            _feat_rows(18, 34, nc.sync)
            _feat_rows(34, 50, nc.sync)
            p3[0].weights()
            _feat_rows(50, 66, nc.sync)
            nc.sync.dma_start(A4r[:, :], x_d[1][:])
            nc.sync.dma_start(A5r[:, :], x_d[2][:])

            def zero_a3():
                _zero_ring(nc, _pad_view(pad0, 0, 64, 64), 64, 64)

            def clspred3():
                _preds_cls(nc, psum_pool, stage_pool, pwc, pbc, A3r, out_d,
                           64, 64, 8, 0, "a")

            _emit_chain(p3, post_hooks={0: zero_a3, 5: clspred3})

            # p4/p5 pass: cls l0 A->B, box l0 A->C, then in-place
            # (towers: cls=B, box=C); p4/p5 layers interleaved for slack
            p4 = [wl(0, 0, A4, B4, 32, 32, "b00"),
                  wl(1, 0, A4, C4, 32, 32, "b10"),
                  wl(0, 1, B4, B4, 32, 32, "b01"),
                  wl(1, 1, C4, C4, 32, 32, "b11"),
                  wl(0, 2, B4, B4, 32, 32, "b02"),
                  wl(1, 2, C4, C4, 32, 32, "b12"),
                  wl(0, 3, B4, B4, 32, 32, "b03"),
                  wl(1, 3, C4, C4, 32, 32, "b13")]
            p5 = [wl(0, 0, A5, B5, 16, 16, "c00"),
                  wl(1, 0, A5, C5, 16, 16, "c10"),
                  wl(0, 1, B5, B5, 16, 16, "c01"),
                  wl(1, 1, C5, C5, 16, 16, "c11"),
                  wl(0, 2, B5, B5, 16, 16, "c02"),
                  wl(1, 2, C5, C5, 16, 16, "c12"),
                  wl(0, 3, B5, B5, 16, 16, "c03"),
                  wl(1, 3, C5, C5, 16, 16, "c13")]
            # first p4/p5 layers' weights + input transforms hide under the
            # p3 box-pred matmuls; their feats were loaded mid-pass-A
            p4[0].weights()
            p5[0].weights()
            p4[0].tf(0)
            p5[0].tf(0)
            _preds_box(nc, psum_pool, stage_pool, pwb, pbb, B3r, out_d,
                       64, 64, 8, 0, "a")

            # p4/p5 scratch rings (pad1/pad2 free after p3 preds)
            _zero_ring(nc, _pad_view(pad1, 0, 32, 32), 32, 32)
            _zero_ring(nc, _pad_view(pad2, 0, 32, 32), 32, 32)
            _zero_ring(nc, _pad_view(pad1, N4, 16, 16), 16, 16)
            _zero_ring(nc, _pad_view(pad2, N4, 16, 16), 16, 16)

            for i in range(8):
                for L in (p4[i], p5[i]):
                    L.weights()
                    L.tf(0)
                    L.mm(0)
                    L.outtf(0)

            _preds_cls(nc, psum_pool, stage_pool, pwc, pbc, B4r, out_d,
                       32, 32, 16, 4096, "b")
            _preds_cls(nc, psum_pool, stage_pool, pwc, pbc, B5r, out_d,
                       16, 16, 16, 5120, "c")
            _preds_box(nc, psum_pool, stage_pool, pwb, pbb, C4r, out_d,
                       32, 32, 16, 4096, "b")
            _preds_box(nc, psum_pool, stage_pool, pwb, pbb, C5r, out_d,
                       16, 16, 16, 5120, "c")

    nc.compile()
    return nc


def _pack_wino_w(wcls, wbox):
    # [s, l, co, ci, ky, kx] -> wino V [s, l, pt, cip, cic, coc, ky, cop]
    w = np.stack([wcls, wbox]).astype(np.float32)   # [2, NL, 256, 256, 3, 3]
    V = np.stack([w[..., 0],
                  (w[..., 0] + w[..., 1] + w[..., 2]) * 0.5,
                  (w[..., 0] - w[..., 1] + w[..., 2]) * 0.5,
                  w[..., 2]], axis=-1)              # [2, NL, co, ci, ky, pt]
    V = V.reshape(2, NL, NCH, P, NCH, P, 3, 4)      # [s,l,coc,cop,cic,cip,ky,pt]
    V = V.transpose(0, 1, 7, 5, 4, 2, 6, 3)         # [s,l,pt,cip,cic,coc,ky,cop]
    return np.ascontiguousarray(V).astype(ml_dtypes.bfloat16)


def _pack_pred_w(w):
    # [co, ci, ky, kx] -> [cip, cic, tap, co]
    n = w.shape[0]
    w = np.asarray(w, np.float32).reshape(n, NCH, P, 3, 3)
    w = w.transpose(2, 1, 3, 4, 0)
    return np.ascontiguousarray(w.reshape(P, NCH, 9, n), dtype=np.float32)


def kernel(p3, p4, p5, stem_cls_w, stem_cls_b, stem_box_w, stem_box_b,
           pred_cls_w, pred_cls_b, pred_box_w, pred_box_b,
           pred_ctr_w, pred_ctr_b):
    if 'nc' not in _cached:
        _cached['nc'] = _build()
    nc = _cached['nc']

    B = p3.shape[0]
    vw = _pack_wino_w(np.asarray(stem_cls_w), np.asarray(stem_box_w))
    sb = np.ascontiguousarray(
        np.stack([stem_cls_b, stem_box_b]).reshape(2, NL, NCH, P, 1),
        dtype=np.float32)
    pwc = _pack_pred_w(np.asarray(pred_cls_w))
    pwb = _pack_pred_w(np.concatenate([pred_box_w, pred_ctr_w], axis=0))
    pbc = np.asarray(pred_cls_b, np.float32).reshape(20, 1)
    pbb = np.concatenate([pred_box_b, pred_ctr_b]).astype(np.float32).reshape(5, 1)

    shared = {"vw": vw, "sb": sb, "pwc": pwc, "pwb": pwb,
              "pbc": pbc, "pbb": pbb}
    xs = [np.asarray(p3, np.float32), np.asarray(p4, np.float32),
          np.asarray(p5, np.float32)]
    in_maps = []
    for b in range(B):
        m = dict(shared)
        for i, x in enumerate(xs):
            xp = np.pad(x[b].reshape(NCH, P, x.shape[2], x.shape[3]),
                        ((0, 0), (0, 0), (1, 1), (1, 1)))
            xp = np.ascontiguousarray(xp.transpose(1, 0, 2, 3))
            m[f"x{i}"] = xp.astype(ml_dtypes.bfloat16) if i == 0 else xp
        in_maps.append(m)

    res = run_bass_kernel_spmd(nc, in_maps, core_ids=list(range(B)),
                               **_run_opts)
    _last['res'] = res
    out = np.stack([r["out"].T for r in res.results])
    return np.ascontiguousarray(out, dtype=np.float32)
